# revision 19
# baseline (speedup 1.0000x reference)
"""Trainium2 Bass kernel for nn_BoxCrossAttention_352187318473.

Math: the reference's attention has a single KV token, so the softmax over
the key axis (length 1) is exactly 1.0 and the output is independent of
x / Wp / Wq / Wk.  The whole module collapses to

    o   = ((mish(y @ W1 + b1) @ W2 + b2)[:, KV:] @ Wv + bv) @ Wo + bo
    out[b, c, w, h] = 9 * o[b, c]          (9 = kernel_size**2 positions)

Everything right of the mish() is LINEAR in m1 = mish(y@W1+b1), so the
weight tail is constant-folded on the host at pack time (weights only --
standard inference-time preprocessing; no activations touched):

    Wbig9  = 9 * (W2[:, KV:] @ Wv @ Wo)            [1024, 256]
    bias9  = 9 * ((b2[KV:] @ Wv + bv) @ Wo + bo)   [256]
    out[b, c, :, :] = m1[b] @ Wbig9[:, c] + bias9[c]

Sharding: output viewed as [B*C, W*H] = [1024, 4096]; core i produces rows
[i*128, (i+1)*128) = batch i//2, channel half i%2.

Per-core schedule (DMA transfers serialize in the cost model; DMA count,
total bytes, and serial dependency latency are what matter):
  - W1 travels as fp8 e3m4 scaled by 32 (read back through an AP bitcast;
    the 1/32 is folded into the sigmoid scale and the final mish multiply),
    halving the critical W1 load.  Measured end-to-end rel err ~1.6e-2 vs
    the 2e-2 gate -- the f16 fallback (W1_FP8=False) measures ~4e-4.
  - b1*32 rides a tiny single-partition [1,1024] DMA and enters PSUM as a
    K=1 lhsT-row matmul per column group, so no separate bias add exists
    and the activation reads PSUM directly.
  - mish is exact via the sigmoid table only:  s = sigmoid(-v),  q = s^2,
    m1 = v*(1-q)/(1+q) = ps * (2r-1)/32 with r = 1/(1+q)  (5 DVE ops).
  - bias9 + PSUM add are folded into the broadcast DVE op:
    bc = ones*bias9 + ps_o  (tensor_scalar, [128, 512] f16).
  - 1 store DMA reads bc through a stride-0 repeat AP ([128, 8, 512]),
    writing the full [128, 4096] f16 shard at full modeled bandwidth.
  - the four framework const-init Pool memsets (verifier-confirmed dead:
    "no reader") are stripped from block 0 so the init barrier doesn't
    wait ~440ns on them.
  - output is stored fp16 and upcast to f32 on the host while unsharding.
"""

import numpy as np

import concourse.bacc as bacc
import concourse.tile as tile
from concourse import mybir
from concourse.bass_utils import run_bass_kernel_spmd

F32 = mybir.dt.float32
F16 = mybir.dt.float16
F8E3 = mybir.dt.float8e3
AF = mybir.ActivationFunctionType
ALU = mybir.AluOpType

B, C, W, H = 4, 256, 64, 64
WH = W * H            # 4096
TAU = 256
KV = 512
N_CORES = 8

W1_FP8 = False         # fp8 e3m4 W1 (x32 scale); False -> f16 W1
BIAS_ROW = False       # b1 via K=1 lhsT-row matmul; False -> DVE add
W1_SCALE = 32.0 if W1_FP8 else 1.0
W1_COLS = 1024 if W1_FP8 else 2048   # f16 columns holding W1 bytes

# fp16 pack1: y k-cols [2] | W1 bytes (fp8 e3m4 x32 or f16, k-major)
PK1_W = 2 + W1_COLS
# fp16 pack2: Wbig9 k-major chunks [8*128] | bias9 f32 col as 2 f16 cols
PK2_W = 8 * 128 + 2
# fp16 aux: b1*32 row on one partition, m-major [1, 1024]
AUX_W = 1024

OUT_DT = F16
BC_W = 512            # broadcast seed width; store repeats it WH//BC_W times

_nc_cache = None


def _strip_const_init_memsets(nc):
    """Drop Bass.__init__'s const-AP Pool memsets (const-float32-1.0 etc.).

    Nothing in this kernel reads them (the BIR verifier flags them as
    reader-less), but they serialize on the Pool engine ahead of the init
    barrier and delay every engine's start by ~440ns.
    """
    dead_names = ("const-float32-1.0", "const-bfloat16-1.0", "const-uint8-127")
    blk0 = nc.main_func.blocks[0]
    dead = []
    for i in blk0.instructions:
        if not (isinstance(i, mybir.InstMemset)
                and i.engine == mybir.EngineType.Pool):
            continue
        out0 = i.outs[0]
        name = getattr(getattr(out0, "tensor", None), "name", "") or str(out0)
        if any(d in str(name) for d in dead_names):
            dead.append(i)
    for i in dead:
        blk0.instructions.remove(i)
    assert len(dead) == 3, f"expected 3 dead const memsets, found {len(dead)}"


def _build_nc():
    nc = bacc.Bacc(trn_type="TRN2")
    _strip_const_init_memsets(nc)

    pk1 = nc.dram_tensor("pk1", [128, PK1_W], F16, kind="ExternalInput")
    pk2 = nc.dram_tensor("pk2", [128, PK2_W], F16, kind="ExternalInput")
    aux = nc.dram_tensor("aux", [1, AUX_W], F16, kind="ExternalInput")
    outd = nc.dram_tensor("out", [128, WH], OUT_DT, kind="ExternalOutput")

    with tile.TileContext(nc) as tc:
        with (
            tc.tile_pool(name="wp", bufs=1) as wp,
            tc.tile_pool(name="ap", bufs=1) as ap,
            tc.tile_pool(name="pp", bufs=1, space="PSUM") as pp,
        ):
            # ones seed: carrier for the broadcast op and rhs for the K=1
            # bias-row matmuls
            ones = ap.tile([128, BC_W], F16, tag="ones")
            nc.vector.memset(ones, 1.0)

            p1 = wp.tile([128, PK1_W], F16, tag="p1")
            nc.sync.dma_start(out=p1, in_=pk1[:, :])
            if BIAS_ROW:
                baux = wp.tile([1, AUX_W], F16, tag="baux")
                nc.sync.dma_start(out=baux, in_=aux[:, :])
            else:
                b1t = wp.tile([128, 8], F16, tag="b1t")
                nc.sync.dma_start(out=b1t, in_=aux[:, :].rearrange(
                    "o (p m) -> (o p) m", p=128))
            p2 = wp.tile([128, PK2_W], F16, tag="p2")
            nc.scalar.dma_start(out=p2, in_=pk2[:, :])

            y_sb = p1[:, 0:2]
            w1r = p1[:, 2:2 + W1_COLS]
            if W1_FP8:
                w1r = w1r.bitcast(F8E3)     # [128, 2048] fp8

            def w1(k, m):                   # W1 block (k,m): [128, 128]
                off = k * 1024 + m * 128
                return w1r[:, off:off + 128]

            def wb(k):                      # Wbig9 k-chunk: [128, 128]
                return p2[:, k * 128:(k + 1) * 128]

            bias9 = p2[:, 1024:1026].bitcast(F32)

            # ---- L1: ps_t1 = (y @ W1 + b1) * 32  (scaled domain) ----
            # 2 K=128 chunks per column group + one K=1 bias-row matmul.
            ps_t1 = pp.tile([128, 8], F32, tag="ps_t1")
            for m in range(8):
                for k in range(2):
                    nc.tensor.matmul(
                        out=ps_t1[:, m:m + 1],
                        lhsT=w1(k, m),
                        rhs=y_sb[:, k:k + 1],
                        start=(k == 0),
                        stop=(k == 1) and not BIAS_ROW,
                    )
                if BIAS_ROW:
                    nc.tensor.matmul(
                        out=ps_t1[:, m:m + 1],
                        lhsT=baux[0:1, m * 128:(m + 1) * 128],
                        rhs=ones[0:1, 0:1],
                        start=False,
                        stop=True,
                    )

            if BIAS_ROW:
                v_in = ps_t1
            else:
                v_in = ap.tile([128, 8], F32, tag="t1b")
                nc.vector.tensor_add(out=v_in, in0=ps_t1, in1=b1t)

            # ---- m1 = mish(v), v = v_in / 32.  Exact via sigmoid table:
            # s = sigmoid(-v), q = s^2, m1 = v*(1-q)/(1+q) = v_in*(2r-1)/32
            s = ap.tile([128, 8], F32, tag="s")
            nc.scalar.activation(out=s, in_=v_in, func=AF.Sigmoid,
                                 scale=-1.0 / W1_SCALE)
            q = ap.tile([128, 8], F32, tag="q")
            nc.vector.tensor_mul(out=q, in0=s, in1=s)
            d = ap.tile([128, 8], F32, tag="d")
            nc.vector.tensor_scalar(out=d, in0=q, scalar1=1.0, scalar2=None,
                                    op0=ALU.add)
            r = ap.tile([128, 8], F32, tag="r")
            nc.vector.reciprocal(out=r, in_=d)
            u = ap.tile([128, 8], F32, tag="u")
            nc.vector.tensor_scalar(out=u, in0=r, scalar1=2.0 / W1_SCALE,
                                    scalar2=-1.0 / W1_SCALE,
                                    op0=ALU.mult, op1=ALU.add)
            m1 = ap.tile([128, 8], F16, tag="m1")
            nc.vector.tensor_mul(out=m1, in0=v_in, in1=u)

            # ---- L2: o[128] = m1 @ Wbig9  (8 k-chunks into one column) ----
            ps_o = pp.tile([128, 1], F32, tag="ps_o")
            for k in range(8):
                nc.tensor.matmul(
                    out=ps_o[:, 0:1],
                    lhsT=wb(k),
                    rhs=m1[:, k:k + 1],
                    start=(k == 0),
                    stop=(k == 7),
                )

            # ---- broadcast seed + store ----
            # bc[p, j] = ones*bias9[p] + ps_o[p]; store repeats it 8x via a
            # stride-0 AP so only BC_W columns are materialized in SBUF.
            bc = ap.tile([128, BC_W], OUT_DT, tag="bc")
            nc.vector.tensor_scalar(
                out=bc, in0=ones, scalar1=bias9, scalar2=ps_o[:, 0:1],
                op0=ALU.mult, op1=ALU.add,
            )
            reps = WH // BC_W
            bc_rep = bc[:, :].unsqueeze(1).broadcast_to((128, reps, BC_W))
            out_v = outd[:, :].rearrange("p (r f) -> p r f", r=reps)
            nc.sync.dma_start(out=out_v, in_=bc_rep)

    return nc


def _host_in_maps(y, W1, b1, W2, b2, Wv, bv, Wo, bo):
    def colpack(mat, kchunks):
        # [K, M] -> [128, kchunks*M], chunk k in cols k*M..(k+1)*M
        K, M = mat.shape
        assert K == kchunks * 128
        return mat.reshape(kchunks, 128, M).transpose(1, 0, 2).reshape(128, -1)

    # host-side weight-tail constant folding (f64 for accuracy)
    Wbig9 = 9.0 * (W2[:, KV:].astype(np.float64) @ Wv.astype(np.float64)
                   @ Wo.astype(np.float64))                       # [1024, 256]
    bias9 = 9.0 * ((b2[KV:].astype(np.float64) @ Wv.astype(np.float64)
                    + bv.astype(np.float64)) @ Wo.astype(np.float64)
                   + bo.astype(np.float64))                       # [256]

    if W1_FP8:
        np_e3 = mybir.dt.np(F8E3)
        w1q = colpack(W1.astype(np.float64) * W1_SCALE, 2).astype(np_e3)
        w1p = np.ascontiguousarray(w1q).view(np.float16)          # [128, 1024]
    else:
        w1p = colpack(W1, 2).astype(np.float16)                   # [128, 2048]

    b1s = b1.astype(np.float64) * W1_SCALE
    if BIAS_ROW:
        auxp = b1s.astype(np.float16)[None, :]              # m-major row
    else:
        # p-major so each SBUF partition reads 8 contiguous elements
        auxp = np.ascontiguousarray(
            b1s.reshape(8, 128).T).astype(np.float16).reshape(1, -1)

    in_maps = []
    for core in range(N_CORES):
        b_i, half = core // 2, core % 2
        ch = slice(half * 128, (half + 1) * 128)
        pk1 = np.empty((128, PK1_W), np.float16)
        pk1[:, 0:2] = y[b_i].reshape(2, 128).T.astype(np.float16)
        pk1[:, 2:] = w1p
        pk2 = np.empty((128, PK2_W), np.float16)
        pk2[:, 0:1024] = colpack(
            np.ascontiguousarray(Wbig9[:, ch]), 8).astype(np.float16)
        pk2[:, 1024:1026] = (
            bias9[ch].astype(np.float32)[:, None].view(np.float16))
        in_maps.append({"pk1": pk1, "pk2": pk2, "aux": auxp})
    return in_maps


def run(inputs, trace=False, **kw):
    global _nc_cache
    if _nc_cache is None:
        _nc_cache = _build_nc()
        _nc_cache.finalize()
    nc = _nc_cache
    in_maps = _host_in_maps(
        np.asarray(inputs["y"], np.float32),
        np.asarray(inputs["W1"], np.float32), np.asarray(inputs["b1"], np.float32),
        np.asarray(inputs["W2"], np.float32), np.asarray(inputs["b2"], np.float32),
        np.asarray(inputs["Wv"], np.float32), np.asarray(inputs["bv"], np.float32),
        np.asarray(inputs["Wo"], np.float32), np.asarray(inputs["bo"], np.float32),
    )
    res = run_bass_kernel_spmd(nc, in_maps, core_ids=list(range(N_CORES)),
                               trace=trace, **kw)
    flat = np.empty((B * C, WH), np.float32)
    for core in range(N_CORES):
        flat[core * 128:(core + 1) * 128] = res.results[core]["out"].astype(np.float32)
    out = flat.reshape(B, C, W, H)
    return out, res


def kernel(**inputs):
    out, _ = run(inputs, trace=False)
    return out
